# revision 9
# baseline (speedup 1.0000x reference)
"""BrainTumorGCNN Trainium2 kernel — Wd-sharded design.

Strategy (8 cores, SPMD, one in-NEFF AllToAll):
  - Core c owns batch c's GCN end-to-end (A^T resident in SBUF as fp8,
    both layers contract via DoubleRow fp8 matmuls).
  - The dense classifier weight Wd [131072, 128] is row-SHARDED across
    cores (16384 rows = 4.2MB bf16 per core) instead of replicated
    (33.5MB per core in the old design) — per-core HBM traffic drops
    ~4x, which is the dominant cost in this memory-bound regime.
  - A 262KB-per-core AllToAll (~6us in-NEFF, measured) redistributes the
    flattened GCN outputs: core r receives, from every batch j, the flat
    rows matching its Wd shard. Each core emits a partial z [8, 128].
  - Host combine: sum the 8 partial z's, then the tiny head
    (relu -> @Wo -> sigmoid) on 8 rows in numpy.
"""

import numpy as np

import concourse.bacc as bacc
import concourse.mybir as mybir
from concourse import tile

B, N, F, H1, H2, D1 = 8, 2048, 128, 32, 64, 128
NCORES = 8
P = 128
MC = N // P             # 16 contraction chunks of 128
AG = 4                  # A chunks per DMA group
NBLK = N // 512         # 4 node blocks of 512
KTOT = N * H2           # 131072 flat rows of Wd
KSH = KTOT // NCORES    # 16384 Wd rows per core
WG = 8                  # wd shard DMA groups
WGC = KSH // P // WG    # 16 chunks of 128 rows per group

REPLICATED = frozenset({"w1", "w2", "b1", "b2"})
BF = mybir.dt.bfloat16
NP_BF = mybir.dt.np(BF)
F8 = mybir.dt.float8e4
NP_F8 = mybir.dt.np(F8)

_cache = {}


def _build(chain=1):
    f32 = mybir.dt.float32
    nc = bacc.Bacc("TRN2", target_bir_lowering=False, debug=False,
                   num_devices=NCORES)

    at_ext = nc.dram_tensor("at", [MC, P, N], F8, kind="ExternalInput")
    xt_ext = nc.dram_tensor("xt", [F, N], BF, kind="ExternalInput")
    w1_ext = nc.dram_tensor("w1", [F, H1], BF, kind="ExternalInput")
    w2_ext = nc.dram_tensor("w2", [H1, H2], BF, kind="ExternalInput")
    b1_ext = nc.dram_tensor("b1", [H1, 1], f32, kind="ExternalInput")
    b2_ext = nc.dram_tensor("b2", [H2, 1], f32, kind="ExternalInput")
    wd_ext = nc.dram_tensor("wd", [WG, P, WGC * P], BF, kind="ExternalInput")
    out_ext = nc.dram_tensor("out", [B, D1], f32, kind="ExternalOutput")

    Relu = mybir.ActivationFunctionType.Relu
    Copy = mybir.ActivationFunctionType.Copy
    DR = mybir.MatmulPerfMode.DoubleRow

    with tile.TileContext(nc) as tc:
        with (
            tc.tile_pool(name="const", bufs=1) as cpool,
            tc.tile_pool(name="amat", bufs=2) as apool,
            tc.tile_pool(name="wd", bufs=2) as wdpool,
            tc.tile_pool(name="work", bufs=1) as wpool,
            tc.tile_pool(name="dram", bufs=2, space="DRAM") as dpool,
            tc.tile_pool(name="ps_t", bufs=2, space="PSUM") as ps_t,
            tc.tile_pool(name="ps_agg", bufs=1, space="PSUM") as ps_a,
            tc.tile_pool(name="ps_z", bufs=1, space="PSUM") as ps_z,
        ):
            xt_sb = cpool.tile([F, N], BF)
            nc.gpsimd.dma_start(xt_sb[:], xt_ext[:])
            w1_sb = cpool.tile([F, H1], BF)
            nc.gpsimd.dma_start(w1_sb[:], w1_ext[:])
            w2_sb = cpool.tile([H1, H2], BF)
            nc.gpsimd.dma_start(w2_sb[:], w2_ext[:])
            b1_sb = cpool.tile([H1, 1], f32)
            nc.gpsimd.dma_start(b1_sb[:], b1_ext[:])
            b2_sb = cpool.tile([H2, 1], f32)
            nc.gpsimd.dma_start(b2_sb[:], b2_ext[:])


            for _it in range(chain):
              # ---- A^T chunks -> SBUF (fp8), one group per DMA queue ----
              a_groups = []
              for g in range(MC // AG):
                  a_g = apool.tile([P, AG * N], F8, tag=f"a{g}")
                  (nc.sync if g % 2 == 0 else nc.scalar).dma_start(
                      a_g[:], at_ext[g * AG:(g + 1) * AG])
                  a_groups.append(a_g)
              a_views = [a_g[:].rearrange("p (k n) -> p k n", k=AG)
                         for a_g in a_groups]

              # ---- Wd shard -> SBUF (bf16, resident), behind A on sync/scalar ----
              wd_tiles = []
              for g in range(WG):
                  wd_g = wdpool.tile([P, WGC * P], BF, tag=f"wd{g}")
                  (nc.sync if g % 2 == 0 else nc.scalar).dma_start(
                      wd_g[:], wd_ext[g])
                  wd_tiles.append(wd_g)

              # ---- t1 = x @ W1 -> fp8 [128, 16*32] (chunk-major) ----
              t1_sb = wpool.tile([P, MC * H1], F8)
              pt1 = ps_t.tile([P, MC * H1], f32, tag="pt")
              for mc in range(MC):
                  nc.tensor.matmul(pt1[:, mc * H1:(mc + 1) * H1],
                                   xt_sb[:, mc * P:(mc + 1) * P],
                                   w1_sb[:], start=True, stop=True)
              nc.scalar.activation(t1_sb[:], pt1[:], Copy)
              t1v = t1_sb[:].rearrange("p (k h) -> p k h", k=MC)

              # ---- h1^T = relu((A @ t1)^T + b1) -> bf16 [32, 2048]
              #      DoubleRow fp8: pairs of contraction chunks ----
              pa1 = [ps_a.tile([H1, 512], f32, tag=f"pagg{nb}", name=f"pa1_{nb}")
                     for nb in range(NBLK)]
              for mp in range(MC // 2):
                  g, i = divmod(2 * mp, AG)
                  for nb in range(NBLK):
                      nc.tensor.matmul(
                          pa1[nb][:],
                          t1v[:, 2 * mp:2 * mp + 2, :],
                          a_views[g][:, i:i + 2, nb * 512:(nb + 1) * 512],
                          start=(mp == 0), stop=(mp == MC // 2 - 1),
                          perf_mode=DR,
                      )
              h1t_sb = wpool.tile([H1, N], BF)
              for nb in range(NBLK):
                  nc.scalar.activation(h1t_sb[:, nb * 512:(nb + 1) * 512],
                                       pa1[nb][:], Relu, bias=b1_sb[:])

              # ---- t2 = h1 @ W2 -> fp8 [128, 16*64] (chunk-major) ----
              t2_sb = wpool.tile([P, MC * H2], F8)
              for half in range(2):
                  pt2 = ps_t.tile([P, 8 * H2], f32, tag="pt")
                  for cc in range(8):
                      mc = half * 8 + cc
                      nc.tensor.matmul(pt2[:, cc * H2:(cc + 1) * H2],
                                       h1t_sb[:, mc * P:(mc + 1) * P],
                                       w2_sb[:], start=True, stop=True)
                  nc.scalar.activation(
                      t2_sb[:, half * 8 * H2:(half + 1) * 8 * H2],
                      pt2[:], Copy)
              t2v = t2_sb[:].rearrange("p (k h) -> p k h", k=MC)

              # ---- flat = relu(A @ t2 + b2) -> bf16 [128, 1024],
              #      column kb holds flat[128*kb : 128*kb+128] ----
              pa2 = [ps_a.tile([H2, 512], f32, tag=f"pagg{nb}", name=f"pa2_{nb}")
                     for nb in range(NBLK)]
              for mp in range(MC // 2):
                  g, i = divmod(2 * mp, AG)
                  for nb in range(NBLK):
                      nc.tensor.matmul(
                          pa2[nb][:],
                          t2v[:, 2 * mp:2 * mp + 2, :],
                          a_views[g][:, i:i + 2, nb * 512:(nb + 1) * 512],
                          start=(mp == 0), stop=(mp == MC // 2 - 1),
                          perf_mode=DR,
                      )
              flat_sb = wpool.tile([P, KTOT // P], BF)
              for nb in range(NBLK):
                  pv = pa2[nb][:].rearrange("c (f two) -> c two f", two=2)
                  nc.scalar.activation(flat_sb[0:H2, nb * 256:(nb + 1) * 256],
                                       pv[:, 0, :], Relu, bias=b2_sb[:])
                  nc.scalar.activation(flat_sb[H2:P, nb * 256:(nb + 1) * 256],
                                       pv[:, 1, :], Relu, bias=b2_sb[:])

              # ---- AllToAll: chunk j of flat (this batch's rows for core
              #      j's Wd shard) -> slot j on core j ----
              bounce_in = dpool.tile([NCORES, P * P], BF, tag="bi")
              bounce_out = dpool.tile([NCORES, P * P], BF, tag="bo")
              for j in range(NCORES):
                  nc.gpsimd.dma_start(bounce_in[j:j + 1, :],
                                      flat_sb[:, j * P:(j + 1) * P])
              nc.gpsimd.collective_compute(
                  "AllToAll", mybir.AluOpType.bypass,
                  replica_groups=[list(range(NCORES))],
                  ins=[bounce_in[:]], outs=[bounce_out[:]],
              )
              flatx_sb = wpool.tile([P, NCORES * P], BF)
              for j in range(NCORES):
                  nc.gpsimd.dma_start(flatx_sb[:, j * P:(j + 1) * P],
                                      bounce_out[j:j + 1, :])
              fxv = flatx_sb[:].rearrange("p (j c) -> p c j", j=NCORES)

              # ---- z_part[j, d] = sum over this core's 16384 Wd rows ----
              zp = ps_z.tile([B, D1], f32)
              for g in range(WG):
                  for cc in range(WGC):
                      c = g * WGC + cc
                      nc.tensor.matmul(
                          zp[:],
                          fxv[:, c, :],
                          wd_tiles[g][:, cc * P:(cc + 1) * P],
                          start=(c == 0), stop=(c == WG * WGC - 1),
                      )
              zo_sb = wpool.tile([B, D1], f32)
              nc.scalar.activation(zo_sb[:], zp[:], Copy)
              nc.gpsimd.dma_start(out_ext[:], zo_sb[:])

    nc.compile()
    return nc


def _make_runner_for(nc):
    return _runner_from_nc(nc)


def _get_runner(chain=1):
    """Cached jitted shard_map executable around the Bass NEFF. chain>1
    repeats the kernel body inside the NEFF for wall-clock timing."""
    key = ("runner", chain)
    if key in _cache:
        return _cache[key]

    nckey = ("nc", chain)
    nc = _cache.get(nckey)
    if nc is None:
        nc = _cache[nckey] = _build(chain)
    runner = _runner_from_nc(nc)
    _cache[key] = runner
    return runner


def _runner_from_nc(nc):
    import jax
    from jax.experimental.shard_map import shard_map
    from jax.sharding import Mesh, PartitionSpec, NamedSharding
    from concourse import bass2jax
    bass2jax.install_neuronx_cc_hook()

    partition_name = nc.partition_id_tensor.name if nc.partition_id_tensor else None
    in_names, out_names, out_avals, zero_outs = [], [], [], []
    for alloc in nc.m.functions[0].allocations:
        if not isinstance(alloc, mybir.MemoryLocationSet):
            continue
        name = alloc.memorylocations[0].name
        if alloc.kind == "ExternalInput":
            if name != partition_name:
                in_names.append(name)
        elif alloc.kind == "ExternalOutput":
            shape = tuple(alloc.tensor_shape)
            dtype = mybir.dt.np(alloc.dtype)
            out_names.append(name)
            out_avals.append(jax.core.ShapedArray(shape, dtype))
            zero_outs.append(np.zeros(shape, dtype))
    n_params = len(in_names)
    n_outs = len(out_avals)
    all_names = in_names + out_names + ([partition_name] if partition_name else [])
    donate = tuple(range(n_params, n_params + n_outs))

    def _body(*args):
        operands = list(args)
        if partition_name is not None:
            operands.append(bass2jax.partition_id_tensor())
        return tuple(bass2jax._bass_exec_p.bind(
            *operands,
            out_avals=tuple(out_avals),
            in_names=tuple(all_names),
            out_names=tuple(out_names),
            lowering_input_output_aliases=(),
            sim_require_finite=True,
            sim_require_nnan=True,
            nc=nc,
        ))

    devices = jax.devices()[:NCORES]
    mesh = Mesh(np.asarray(devices), ("core",))
    in_specs = tuple(
        PartitionSpec() if name in REPLICATED else PartitionSpec("core")
        for name in in_names) + (PartitionSpec("core"),) * n_outs
    fn = jax.jit(
        shard_map(_body, mesh=mesh, in_specs=in_specs,
                  out_specs=(PartitionSpec("core"),) * n_outs,
                  check_rep=False),
        donate_argnums=donate, keep_unused=True,
    )
    shardings = {
        name: NamedSharding(mesh, PartitionSpec() if name in REPLICATED
                            else PartitionSpec("core"))
        for name in in_names}
    return {
        "fn": fn, "in_names": in_names, "out_names": out_names,
        "zero_outs": zero_outs, "mesh": mesh,
        "sharding": NamedSharding(mesh, PartitionSpec("core")),
        "shardings": shardings,
        "out_avals": out_avals,
    }


def _prep(x, a, W1, b1, W2, b2, Wd, bd, Wo, bo):
    """Host-side shard/layout prep -> dict of concatenated (8*dim0) inputs.
    (bd/Wo/bo are applied host-side in kernel(); not shipped to device.)"""
    x = np.asarray(x, np.float32)
    a = np.asarray(a, np.float32)
    W1 = np.ascontiguousarray(np.asarray(W1, NP_BF))
    W2 = np.ascontiguousarray(np.asarray(W2, NP_BF))
    b1c = np.asarray(b1, np.float32).reshape(H1, 1)
    b2c = np.asarray(b2, np.float32).reshape(H2, 1)
    Wd = np.asarray(Wd, np.float32)

    at = np.ascontiguousarray(
        a.astype(NP_F8).transpose(0, 2, 1)).reshape(NCORES * MC, P, N)
    xt = np.ascontiguousarray(
        x.astype(NP_BF).transpose(0, 2, 1)).reshape(NCORES * F, N)
    # Wd row-shard: core c gets rows [16384c, 16384(c+1)) as
    # [WG, P, WGC*P] with wd[g][p, cc*128+d] = Wd[16384c+(16g+cc)*128+p, d]
    wdq = np.ascontiguousarray(
        Wd.astype(NP_BF).reshape(NCORES, WG, WGC, P, D1)
        .transpose(0, 1, 3, 2, 4).reshape(NCORES * WG, P, WGC * P))

    return {
        "at": at, "xt": xt, "w1": W1, "w2": W2, "b1": b1c,
        "b2": b2c, "wd": wdq,
    }


def _run(runner, concat_ins):
    args = [concat_ins[name] for name in runner["in_names"]]
    zeros = [np.zeros((NCORES * z.shape[0], *z.shape[1:]), z.dtype)
             for z in runner["zero_outs"]]
    return runner["fn"](*args, *zeros)


def kernel(x, a, W1, b1, W2, b2, Wd, bd, Wo, bo):
    runner = _get_runner()
    concat_ins = _prep(x, a, W1, b1, W2, b2, Wd, bd, Wo, bo)
    outs = _run(runner, concat_ins)
    oi = runner["out_names"].index("out")
    # [NCORES*8, 128]: core c's rows are its partial z for all 8 batches
    zparts = np.asarray(outs[oi]).reshape(NCORES, B, D1).astype(np.float32)
    z = zparts.sum(axis=0)
    bd = np.asarray(bd, np.float32).reshape(1, D1)
    Wo = np.asarray(Wo, np.float32).reshape(D1, 1)
    bo = np.asarray(bo, np.float32).reshape(1, 1)
    logit = np.clip(np.maximum(z + bd, 0.0) @ Wo + bo, -30.0, 30.0)
    return (1.0 / (1.0 + np.exp(-logit))).astype(np.float32)
